# revision 7
# baseline (speedup 1.0000x reference)
"""Chamfer 1D loss on 8 TRN2 NeuronCores.

Sharding: core c owns x[2048c:2048(c+1)] and y[2048c:2048(c+1)] as "row"
blocks; each direction's min is computed against the FULL other array
(replicated to every core), so no inter-core collective is needed — each
core emits partial row-minima and the host sums them.

Per core, per direction: rows [2048] live one-per-partition-lane as 16
tiles of [128,1]; the full opposing array [16384] is partition-broadcast
into SBUF. One fused 2-stream custom DVE instruction per (chunk, tile)
reads TWO disjoint column half-chunks on the DVE's two SBUF read ports
(in0/in1), computes min(|in0 - row|, |in1 - row|) per element and
min-reduces it (exact fp32) into one cell of the row-minima tile — so
each DVE cycle retires TWO (row, col) pairs, 2x the single-stream rate.
Output per core: [2, 128, 16] row minima; host sums.
"""

import numpy as np

import concourse.bacc as bacc
import concourse.mybir as mybir
import concourse.tile as tile
import concourse.bass_utils as bass_utils

import concourse.bass_isa as bass_isa
import concourse.dve_ops as dve_ops
from concourse.dve_ops import DveOp, get_dve_sub_opcode
from concourse.dve_spec import (
    Spec, Src0, Src1, C0, C1, Zero, maxx, minn, lower, _has_src1,
)
from concourse.dve_uop import DveOpSpec

F32 = mybir.dt.float32
P = 128          # partitions
NF = 16384       # full length of each input
NB = NF // 8     # row block per core (2048)
T = NB // P      # row tiles per block (16)
# Column chunks per direction (each chunk is split in half across the two
# DVE read ports, so an instruction's free-dim is chunk/2). Direction 0
# streams a small first chunk so the first instruction starts early;
# direction 1 uses one big chunk whose broadcast DMA overlaps direction-0
# compute.
CHUNKS = [[512, 3072, 12800], [16384]]
ALPHA = 0.5
BIG = 3.0e38

OP_NAME = "CHAMFER_ABSMIN2_ANT"


def _ref(in0, in1, s0, s1, imm2):
    a = np.abs(in0.astype(np.float32) - s0)
    b = np.abs(in1.astype(np.float32) - s0)
    body = np.minimum(a, b).astype(np.float32)
    seed = s1 if isinstance(s1, (int, float)) else np.asarray(s1, np.float32)
    flat = body.reshape(body.shape[0], -1)
    acc = np.minimum(flat.min(axis=-1, keepdims=True), seed)
    if isinstance(acc, np.ndarray):
        acc = acc.reshape(body.shape[0], 1)
    return body, acc


def _register_op() -> DveOp:
    if OP_NAME in dve_ops._SUB_OPCODE_FOR_NAME:
        for op in dve_ops.OPS:
            if op.name == OP_NAME:
                return op
        raise RuntimeError("row allocated but op missing")
    d0 = Src0 - C0
    d1 = Src1 - C0
    spec = Spec(
        body=minn(maxx(d0, Zero - d0), maxx(d1, Zero - d1)),
        accum=minn,
        accum_init=C1,
        reference=_ref,
    )
    row = dve_ops._CUSTOM_DVE_ROW_BASE + len(dve_ops.OPS)
    assert row < 0x20
    dve_ops._SUB_OPCODE_FOR_NAME[OP_NAME] = row
    shas = {}
    for ver in ("v3", "v4"):
        shas[ver] = DveOpSpec(
            name=OP_NAME, opcode=row, uops=lower(spec, ver=ver),
            rd1_en=_has_src1(spec),
        ).sha(ver)
    op = DveOp(OP_NAME, spec, subdim=False, uops_sha=shas)
    dve_ops.OPS.append(op)
    dve_ops.CUSTOM_DVE_SPECS[OP_NAME] = spec
    return op


CHAMFER_OP = _register_op()


def _emit_chamfer2(vec, *, out, in0, in1, s0, s1, accum_out):
    """_custom_dve replica that skips AP optimization on `out` so a step-0
    broadcast write pattern (scratch-free body sink) survives lowering."""
    op = CHAMFER_OP
    bassm = vec.bass
    if op.name not in bassm.m.ant_custom_dve_ops:
        bassm.m.ant_custom_dve_ops = sorted({*bassm.m.ant_custom_dve_ops, op.name})
    op.compile("v3")
    shape = bass_isa.CustomDveShape.TTSS
    opc = bassm.isa.Opcode[
        f"NEURON_ISA_TPB_OPCODE_CUSTOM_DVE_ANT_{shape.slot()}"
    ].value

    def lower_scalar(v):
        if isinstance(v, (int, float)):
            return mybir.ImmediateValue(dtype=mybir.dt.float32, value=float(v))
        return vec.lower_ap(v, for_isa=True)

    ins_l = [
        vec.lower_ap(in0, for_isa=True, opt=True),
        vec.lower_ap(in1, for_isa=True, opt=True),
        lower_scalar(s0),
        lower_scalar(s1),
    ]
    outs_l = [
        vec.lower_ap(out, for_isa=True, opt=False),
        vec.lower_ap(accum_out, for_isa=True),
    ]
    return vec.add_instruction(
        bass_isa.InstCustomDveAnt(
            name=bassm.get_next_instruction_name(),
            op_name=op.name,
            rd1_en=True,
            subdim=0,
            imm2=0.0,
            shape=shape,
            row=get_dve_sub_opcode(op.name),
            isa_opcode=opc,
            ins=ins_l,
            outs=outs_l,
        )
    )


_NC_CACHE = None


def _build():
    global _NC_CACHE
    if _NC_CACHE is not None:
        return _NC_CACHE
    nc = bacc.Bacc("TRN2", target_bir_lowering=False, debug=False, num_devices=8)
    x_blk = nc.dram_tensor("x_blk", [NB], F32, kind="ExternalInput")
    y_blk = nc.dram_tensor("y_blk", [NB], F32, kind="ExternalInput")
    x_full = nc.dram_tensor("x_full", [NF], F32, kind="ExternalInput")
    y_full = nc.dram_tensor("y_full", [NF], F32, kind="ExternalInput")
    mins = nc.dram_tensor("mins", [2, P, T], F32, kind="ExternalOutput")

    with tile.TileContext(nc) as tc:
        with (
            tc.tile_pool(name="bcast0", bufs=1) as bc_pool0,
            tc.tile_pool(name="bcast1", bufs=1) as bc_pool1,
            tc.tile_pool(name="scratch", bufs=2) as sc_pool,
            tc.tile_pool(name="small", bufs=1) as small,
        ):
            for d, (rows_dram, cols_dram) in enumerate(
                [(x_blk, y_full), (y_blk, x_full)]
            ):
                chunks = CHUNKS[d]
                bc_pool = bc_pool0 if d == 0 else bc_pool1
                # rows: [128, T], partition p / tile t <- rows_dram[p*T + t]
                rows_sb = small.tile([P, T], F32, tag=f"rows{d}")
                # tiny strided row loads ride the ACT ring so they don't
                # serialize ahead of the first broadcast on the SP ring
                nc.scalar.dma_start(
                    rows_sb[:], rows_dram.ap().rearrange("(p t) -> p t", p=P)
                )
                # row minima, carry-chained across chunks via the accum seed
                minw = small.tile([P, T], F32, tag=f"minw{d}")
                # pre-issue every broadcast DMA (all on the SP HWDGE ring;
                # the ACT ring measured ~20% slower for these broadcasts)
                cols_tiles = []
                off = 0
                for ch, cw in enumerate(chunks):
                    cols_sb = bc_pool.tile([P, cw], F32, tag=f"cols{d}c{ch}")
                    nc.sync.dma_start(
                        cols_sb[:],
                        cols_dram.ap()[off : off + cw]
                        .unsqueeze(0)
                        .partition_broadcast(P),
                    )
                    cols_tiles.append(cols_sb)
                    off += cw
                for ch, cw in enumerate(chunks):
                    cols_sb = cols_tiles[ch]
                    h = cw // 2
                    for t in range(T):
                        # body values are scratch: sink every write into one
                        # cell via a step-0 broadcast AP (no big scratch tile)
                        scr = sc_pool.tile([P, 1], F32, tag="scr")
                        _emit_chamfer2(
                            nc.vector,
                            out=scr[:, 0:1].to_broadcast([P, h]),
                            in0=cols_sb[:, 0:h],
                            in1=cols_sb[:, h:cw],
                            s0=rows_sb[:, t : t + 1],
                            s1=BIG if ch == 0 else minw[:, t : t + 1],
                            accum_out=minw[:, t : t + 1],
                        )
                nc.sync.dma_start(mins.ap()[d], minw[:])
    nc.compile()
    _NC_CACHE = nc
    return nc


def kernel(**inputs: np.ndarray) -> np.ndarray:
    x = np.ascontiguousarray(inputs["inputs"], dtype=np.float32).reshape(-1)
    y = np.ascontiguousarray(inputs["targets"], dtype=np.float32).reshape(-1)
    assert x.shape == (NF,) and y.shape == (NF,)

    nc = _build()
    in_maps = [
        {
            "x_blk": x[c * NB : (c + 1) * NB],
            "y_blk": y[c * NB : (c + 1) * NB],
            "x_full": x,
            "y_full": y,
        }
        for c in range(8)
    ]
    res = bass_utils.run_bass_kernel_spmd(nc, in_maps, core_ids=list(range(8)))

    cd_xy = 0.0
    cd_yx = 0.0
    for c in range(8):
        m = res.results[c]["mins"]
        cd_xy += m[0].sum(dtype=np.float64)
        cd_yx += m[1].sum(dtype=np.float64)
    val = ALPHA * cd_xy / NF + (1.0 - ALPHA) * cd_yx / NF
    return np.float32(val)


# revision 9
# speedup vs baseline: 1.0040x; 1.0040x over previous
"""Chamfer 1D loss on 8 TRN2 NeuronCores.

Sharding: core c owns x[2048c:2048(c+1)] and y[2048c:2048(c+1)] as "row"
blocks; each direction's min is computed against the FULL other array
(replicated to every core), so no inter-core collective is needed — each
core emits partial row-minima and the host sums them.

Per core, per direction: rows [2048] live one-per-partition-lane as 16
tiles of [128,1]; the full opposing array [16384] is partition-broadcast
into SBUF. One fused 2-stream custom DVE instruction per (chunk, tile)
reads TWO disjoint column half-chunks on the DVE's two SBUF read ports
(in0/in1), computes min(|in0 - row|, |in1 - row|) per element and
min-reduces it (exact fp32) into one cell of the row-minima tile — so
each DVE cycle retires TWO (row, col) pairs, 2x the single-stream rate.
Output per core: [2, 128, 16] row minima; host sums.
"""

import numpy as np

import concourse.bacc as bacc
import concourse.mybir as mybir
import concourse.tile as tile
import concourse.bass_utils as bass_utils

import concourse.bass_isa as bass_isa
import concourse.dve_ops as dve_ops
from concourse.dve_ops import DveOp, get_dve_sub_opcode
from concourse.dve_spec import (
    Spec, Src0, Src1, C0, C1, Zero, maxx, minn, lower, _has_src1,
)
from concourse.dve_uop import DveOpSpec

F32 = mybir.dt.float32
P = 128          # partitions
NF = 16384       # full length of each input
NB = NF // 8     # row block per core (2048)
T = NB // P      # row tiles per block (16)
# Column chunks per direction (each chunk is split in half across the two
# DVE read ports, so an instruction's free-dim is chunk/2). Direction 0
# streams a small first chunk so the first instruction starts early;
# direction 1 uses one big chunk whose broadcast DMA overlaps direction-0
# compute.
CHUNKS = [[512, 3072, 12800], [16384]]
ALPHA = 0.5
BIG = 3.0e38

OP_NAME = "CHAMFER_ABSMIN2_ANT"


def _ref(in0, in1, s0, s1, imm2):
    a = np.abs(in0.astype(np.float32) - s0)
    b = np.abs(in1.astype(np.float32) - s0)
    body = np.minimum(a, b).astype(np.float32)
    seed = s1 if isinstance(s1, (int, float)) else np.asarray(s1, np.float32)
    flat = body.reshape(body.shape[0], -1)
    acc = np.minimum(flat.min(axis=-1, keepdims=True), seed)
    if isinstance(acc, np.ndarray):
        acc = acc.reshape(body.shape[0], 1)
    return body, acc


def _register_op() -> DveOp:
    if OP_NAME in dve_ops._SUB_OPCODE_FOR_NAME:
        for op in dve_ops.OPS:
            if op.name == OP_NAME:
                return op
        raise RuntimeError("row allocated but op missing")
    d0 = Src0 - C0
    d1 = Src1 - C0
    spec = Spec(
        body=minn(maxx(d0, Zero - d0), maxx(d1, Zero - d1)),
        accum=minn,
        accum_init=C1,
        reference=_ref,
    )
    row = dve_ops._CUSTOM_DVE_ROW_BASE + len(dve_ops.OPS)
    assert row < 0x20
    dve_ops._SUB_OPCODE_FOR_NAME[OP_NAME] = row
    shas = {}
    for ver in ("v3", "v4"):
        shas[ver] = DveOpSpec(
            name=OP_NAME, opcode=row, uops=lower(spec, ver=ver),
            rd1_en=_has_src1(spec),
        ).sha(ver)
    op = DveOp(OP_NAME, spec, subdim=False, uops_sha=shas)
    dve_ops.OPS.append(op)
    dve_ops.CUSTOM_DVE_SPECS[OP_NAME] = spec
    return op


CHAMFER_OP = _register_op()


def _emit_chamfer2(vec, *, out, in0, in1, s0, s1, accum_out):
    """_custom_dve replica that skips AP optimization on `out` so a step-0
    broadcast write pattern (scratch-free body sink) survives lowering."""
    op = CHAMFER_OP
    bassm = vec.bass
    if op.name not in bassm.m.ant_custom_dve_ops:
        bassm.m.ant_custom_dve_ops = sorted({*bassm.m.ant_custom_dve_ops, op.name})
    op.compile("v3")
    shape = bass_isa.CustomDveShape.TTSS
    opc = bassm.isa.Opcode[
        f"NEURON_ISA_TPB_OPCODE_CUSTOM_DVE_ANT_{shape.slot()}"
    ].value

    def lower_scalar(v):
        if isinstance(v, (int, float)):
            return mybir.ImmediateValue(dtype=mybir.dt.float32, value=float(v))
        return vec.lower_ap(v, for_isa=True)

    ins_l = [
        vec.lower_ap(in0, for_isa=True, opt=True),
        vec.lower_ap(in1, for_isa=True, opt=True),
        lower_scalar(s0),
        lower_scalar(s1),
    ]
    outs_l = [
        vec.lower_ap(out, for_isa=True, opt=False),
        vec.lower_ap(accum_out, for_isa=True),
    ]
    return vec.add_instruction(
        bass_isa.InstCustomDveAnt(
            name=bassm.get_next_instruction_name(),
            op_name=op.name,
            rd1_en=True,
            subdim=0,
            imm2=0.0,
            shape=shape,
            row=get_dve_sub_opcode(op.name),
            isa_opcode=opc,
            ins=ins_l,
            outs=outs_l,
        )
    )


_NC_CACHE = None


def _build():
    global _NC_CACHE
    if _NC_CACHE is not None:
        return _NC_CACHE
    nc = bacc.Bacc("TRN2", target_bir_lowering=False, debug=False, num_devices=8)
    x_blk = nc.dram_tensor("x_blk", [NB], F32, kind="ExternalInput")
    y_blk = nc.dram_tensor("y_blk", [NB], F32, kind="ExternalInput")
    x_full = nc.dram_tensor("x_full", [NF], F32, kind="ExternalInput")
    y_full = nc.dram_tensor("y_full", [NF], F32, kind="ExternalInput")
    mins = nc.dram_tensor("mins", [2, P, T], F32, kind="ExternalOutput")

    with tile.TileContext(nc) as tc:
        with (
            tc.tile_pool(name="bcast0", bufs=1) as bc_pool0,
            tc.tile_pool(name="bcast1", bufs=1) as bc_pool1,
            tc.tile_pool(name="scratch", bufs=2) as sc_pool,
            tc.tile_pool(name="small", bufs=1) as small,
        ):
            for d, (rows_dram, cols_dram) in enumerate(
                [(x_blk, y_full), (y_blk, x_full)]
            ):
                chunks = CHUNKS[d]
                bc_pool = bc_pool0 if d == 0 else bc_pool1
                # rows: [128, T], partition p / tile t <- rows_dram[p*T + t]
                rows_sb = small.tile([P, T], F32, tag=f"rows{d}")
                # tiny strided row loads ride the ACT ring so they don't
                # serialize ahead of the first broadcast on the SP ring
                nc.scalar.dma_start(
                    rows_sb[:],
                    rows_dram.ap().rearrange("(p t) -> p t", p=P),
                    single_packet=True,
                )
                # row minima, carry-chained across chunks via the accum seed
                minw = small.tile([P, T], F32, tag=f"minw{d}")
                # pre-issue every broadcast DMA (all on the SP HWDGE ring;
                # the ACT ring measured ~20% slower for these broadcasts)
                cols_tiles = []
                off = 0
                for ch, cw in enumerate(chunks):
                    cols_sb = bc_pool.tile([P, cw], F32, tag=f"cols{d}c{ch}")
                    # the small starter chunk goes out as one packet so its
                    # completion isn't stretched by round-robin with the big
                    # broadcasts on the same ring
                    nc.sync.dma_start(
                        cols_sb[:],
                        cols_dram.ap()[off : off + cw]
                        .unsqueeze(0)
                        .partition_broadcast(P),
                        single_packet=(d == 0 and ch == 0),
                    )
                    cols_tiles.append(cols_sb)
                    off += cw
                for ch, cw in enumerate(chunks):
                    cols_sb = cols_tiles[ch]
                    h = cw // 2
                    for t in range(T):
                        # body values are scratch: sink every write into one
                        # cell via a step-0 broadcast AP (no big scratch tile)
                        scr = sc_pool.tile([P, 1], F32, tag="scr")
                        _emit_chamfer2(
                            nc.vector,
                            out=scr[:, 0:1].to_broadcast([P, h]),
                            in0=cols_sb[:, 0:h],
                            in1=cols_sb[:, h:cw],
                            s0=rows_sb[:, t : t + 1],
                            s1=BIG if ch == 0 else minw[:, t : t + 1],
                            accum_out=minw[:, t : t + 1],
                        )
                nc.sync.dma_start(mins.ap()[d], minw[:])
    nc.compile()
    _NC_CACHE = nc
    return nc


def kernel(**inputs: np.ndarray) -> np.ndarray:
    x = np.ascontiguousarray(inputs["inputs"], dtype=np.float32).reshape(-1)
    y = np.ascontiguousarray(inputs["targets"], dtype=np.float32).reshape(-1)
    assert x.shape == (NF,) and y.shape == (NF,)

    nc = _build()
    in_maps = [
        {
            "x_blk": x[c * NB : (c + 1) * NB],
            "y_blk": y[c * NB : (c + 1) * NB],
            "x_full": x,
            "y_full": y,
        }
        for c in range(8)
    ]
    res = bass_utils.run_bass_kernel_spmd(nc, in_maps, core_ids=list(range(8)))

    cd_xy = 0.0
    cd_yx = 0.0
    for c in range(8):
        m = res.results[c]["mins"]
        cd_xy += m[0].sum(dtype=np.float64)
        cd_yx += m[1].sum(dtype=np.float64)
    val = ALPHA * cd_xy / NF + (1.0 - ALPHA) * cd_yx / NF
    return np.float32(val)
